# revision 19
# baseline (speedup 1.0000x reference)
"""MoE FFN (8 experts, top-2) on 8 Trainium2 NeuronCores.

Strategy: expert-parallel. The router (tiny: T x D @ D x E, 0.05% of the
FLOPs) runs on host, tokens are dispatched (gathered + padded) per expert
on host, and core e runs the SwiGLU FFN for expert e over its tokens:
    y = (silu(x @ Wg[e]) * (x @ Wu[e])) @ Wd[e] * combine_weight
Per-core capacity is fixed at C = T*top_k/E = 2048 token-pairs (the
perfectly balanced load); the few overflow pairs of overloaded experts
(~1.8% for near-uniform routing) are computed on host in fp32. The
per-(token,expert) outputs are un-permuted and summed over the top-2
assignments on host. All matmuls run in bf16 with fp32 PSUM accumulation
(measured 0.4% relative error end-to-end).

On-device layout per core (all stationary operands f/d-chunked to 128):
  stage 1: G^T/U^T [f,t] tiles = Wg/Wu chunk^T @ X^T, f-major so stage 2
           needs no transpose; silu+mul fused on scalar/vector engines.
  stage 2: Y [t,d] = H^T chunks^T @ Wd chunks, scaled by combine weight.

Shapes are hardcoded for B=4, S=2048, D=1024, F=2816, E=8, top_k=2.

Performance: the per-core program issues 2112 N=512 bf16 matmuls =
1.08M PE streaming cycles, the exact MAC-count floor for a 128x128
array. Cost-model sim: 466us total with PE 96.9% busy and a 99.7%-dense
PE stream (the 3% = DMA lead-in + drain tail, both amortized across
invocations). The HW/sim gap is P0 clock throttle under 8-core load
(session-dependent, ~2.0-2.2 GHz vs the 2.4 GHz peak).

Measurement notes (this shapes test.py): each For_i back-edge costs
~10-15us on HW even with staggered_reset, so benchmark programs must
unroll bodies (repeats x unroll) — the old 42x1-loop protocol
over-reported per-body time by ~2.5%. With back-edge-free bodies,
burst (42-body window) and sustained (102-body) estimates agree to
~1-2%. Hoisting the 17MB weight re-DMA out of the benchmark loop
measures identically (+-0.3%) — DMA power is not the throttle driver.

Precision: fp8e4/e5 + DoubleRow is the only sub-bf16-cycle mode on
TRN2, but both operands quantize to 3 mantissa bits: measured end-to-
end L2 error 6.5% all-fp8, 3.8% for the rank-2 half of the pairs —
over the 2e-2 gate. The shipped compromise: a 256-slot fp8 sub-tile
per core (_C8) running gate/up in DoubleRow (half the PE cycles on
12.5% of slots, down-projection bf16), fed the per-expert smallest-
combine-weight pairs so the quantization error lands where the combine
weight minimizes it. fp8 weights are streamed from HBM per f-chunk
(used once per body — no SBUF residency needed). Device-measured:
rel err 1.16e-2, ~10us/iter faster (~2%; half the ideal 22us because
DoubleRow disables FWL, leaving the 256-col stationary loads partially
LDWEIGHTS-bound at N=256 moving). Wider fp8 coverage (C8=512) would
fix the LDW ratio but pushes error to ~2e-2 = the gate. The bf16 bulk
runs at ~100% of the throttled tensor-engine roofline (a pure-matmul
probe of the same stream measures identical time).
"""

import numpy as np
import ml_dtypes

import concourse.mybir as mybir
import concourse.tile as tile
from concourse import bacc
from concourse.bass_utils import run_bass_kernel_spmd

BF16 = ml_dtypes.bfloat16

D = 1024
F = 2816
E = 8
TOPK = 2
DC = D // 128  # 8 contraction chunks for gate/up
FC = F // 128  # 22 contraction chunks for down

# Slots per core computed with fp8-e4m3 DoubleRow gate/up matmuls (the last
# C8 of the 2048): the host routes each expert's smallest-combine-weight
# pairs there, so the fp8 quantization error lands on the least-weighted
# pairs. Down-projection stays bf16. Measured end-to-end rel err ~1.3e-2
# (vs 4.1e-3 all-bf16, gate 2e-2); saves half the gate/up PE cycles on
# C8/C of the slots.
_C8 = 256

# Options for the benchmark-only repeats loop (see _build). staggered_reset
# replaces the default drain + all-engine-barrier back-edge (~2us, plus it
# serializes each iteration's weight-DMA ramp) with overlapped per-stage
# semaphore resets; measured ~10us/iter faster than the barrier back-edge.
_LOOP_OPTS: dict = {"staggered_reset": True}


def _route(x, Wr):
    """Host router matching the jax reference: softmax -> top-2 -> renorm.

    top_k on probs == top_k on logits (softmax is monotone); argsort with
    stable kind matches jax.lax.top_k's lowest-index tie-break.
    """
    logits = x @ Wr  # [T, E] fp32
    lmax = logits.max(-1, keepdims=True)
    p = np.exp(logits - lmax)
    p /= p.sum(-1, keepdims=True)
    idx = np.argsort(-p, axis=-1, kind="stable")[:, :TOPK]
    w = np.take_along_axis(p, idx, -1)
    w = w / w.sum(-1, keepdims=True)
    return idx.astype(np.int64), w.astype(np.float32)


def _token_tiles(C):
    tiles = []
    t0 = 0
    while t0 < C:
        tt = min(512, C - t0)
        tiles.append((t0, tt))
        t0 += tt
    return tiles


def _build(C, repeats=1, w_outside=False, unroll=1):
    """Build + compile the per-core expert-FFN program for capacity C.

    repeats>1 wraps the body in a hardware loop (body runs repeats*unroll
    times) — used only for wall-clock benchmarking (per-iteration time =
    delta / extra iterations). w_outside hoists the iteration-invariant
    weight + combine-weight DMAs out of the benchmark loop; unroll>1 emits
    several bodies per hardware-loop iteration (fewer back-edges).
    """
    f32 = mybir.dt.float32
    bf16 = mybir.dt.bfloat16
    AF = mybir.ActivationFunctionType

    nc = bacc.Bacc("TRN2", target_bir_lowering=False, debug=False, num_devices=E)

    wg_d = nc.dram_tensor("wg", [D, F], bf16, kind="ExternalInput").ap()
    wu_d = nc.dram_tensor("wu", [D, F], bf16, kind="ExternalInput").ap()
    wd_d = nc.dram_tensor("wd", [F, D], bf16, kind="ExternalInput").ap()
    fp8 = mybir.dt.float8e4
    G = C // 128  # token groups of 128
    C8 = _C8 if C == 2048 else 0  # fp8-DoubleRow sub-tile slots (see kernel())
    CB = C - C8
    xt_d = nc.dram_tensor("xt", [D, C], bf16, kind="ExternalInput").ap()
    # combine weights pre-packed on host as [128, G]: column g holds the
    # weights of token group g (one straight DMA instead of a 2048x4B
    # descriptor gather, which used to stall a DMA queue)
    wts_d = nc.dram_tensor("wts", [128, G], f32, kind="ExternalInput").ap()
    if C8:
        # fp8 operands for the DoubleRow sub-tile: x pre-scaled by 8,
        # gate/up weights pre-scaled by 256 (exact powers of two; the
        # 1/2048 descale folds into the activation scale and the host-
        # prepared combine weights of these slots). Host pre-packs the
        # SBUF partition layouts: xt8 [p, c*C8+n] = X8[c*128+p, n];
        # w*8 [p, f*1024 + q*128 + m] = W8[q*128+p, f*128+m].
        xt8_d = nc.dram_tensor(
            "xt8", [128, DC * C8], fp8, kind="ExternalInput"
        ).ap()
        wg8_d = nc.dram_tensor(
            "wg8", [128, FC * DC * 128], fp8, kind="ExternalInput"
        ).ap()
        wu8_d = nc.dram_tensor(
            "wu8", [128, FC * DC * 128], fp8, kind="ExternalInput"
        ).ap()
    y_d = nc.dram_tensor("y", [C, D], f32, kind="ExternalOutput").ap()

    import contextlib

    with tile.TileContext(nc) as tc:
        with contextlib.ExitStack() as stk:
            wpool = stk.enter_context(tc.tile_pool(name="weights", bufs=1))
            xpool = stk.enter_context(tc.tile_pool(name="xt", bufs=2))
            hpool = stk.enter_context(tc.tile_pool(name="ht", bufs=1))
            spool = stk.enter_context(tc.tile_pool(name="silu", bufs=3))
            ypool = stk.enter_context(tc.tile_pool(name="yout", bufs=3))
            pg_pool = stk.enter_context(tc.tile_pool(name="pg", bufs=2, space="PSUM"))
            pu_pool = stk.enter_context(tc.tile_pool(name="pu", bufs=2, space="PSUM"))
            py_pool = stk.enter_context(tc.tile_pool(name="py", bufs=4, space="PSUM"))
            if C8:
                # double-buffered stream of fp8 gate/up weight columns (each
                # [128, DC*128] chunk is used once per body; no residency)
                wspool = stk.enter_context(tc.tile_pool(name="w8", bufs=2))

            def emit_weights():
                # Resident weights: gate/up as [128, DC*F] (d-chunk major),
                # down as [128, FC*D] (f-chunk major). Column-chunked DMA so
                # the first f-chunks of stage 1 are ready before the full
                # 17MB lands.
                wg_sb = wpool.tile([128, DC * F], bf16, tag="wg")
                wu_sb = wpool.tile([128, DC * F], bf16, tag="wu")
                wd_sb = wpool.tile([128, FC * D], bf16, tag="wd")
                CCH = 4  # column chunks per [128, F] weight slice
                for c0, c1 in [
                    (F * c // CCH, F * (c + 1) // CCH) for c in range(CCH)
                ]:
                    for d in range(DC):
                        nc.sync.dma_start(
                            wg_sb[:, d * F + c0:d * F + c1],
                            wg_d[d * 128:(d + 1) * 128, c0:c1],
                        )
                    for d in range(DC):
                        nc.sync.dma_start(
                            wu_sb[:, d * F + c0:d * F + c1],
                            wu_d[d * 128:(d + 1) * 128, c0:c1],
                        )
                # Wd in two ~2.9MB DMAs (f-chunk-major per partition); the
                # first half completes early enough for tile 0's stage 2.
                wd_3d = wd_d.rearrange("(f p) d -> p f d", p=128)
                wd_sb3 = wd_sb[:].rearrange("p (f d) -> p f d", f=FC)
                for fa, fb in ((0, FC // 2), (FC // 2, FC)):
                    nc.sync.dma_start(wd_sb3[:, fa:fb], wd_3d[:, fa:fb])
                return wg_sb, wu_sb, wd_sb

            def emit_wts():
                # Combine weights, one column per 128-token group,
                # host-packed as [128, G] so this is one contiguous DMA.
                # Issued last: first use is ~90us in (stage 2, tile 0).
                wts_sb = wpool.tile([128, G], f32, tag="wts")
                nc.sync.dma_start(wts_sb[:], wts_d)
                return wts_sb

            def emit_body(first, weights):
                if weights is None:
                    # Token tile 0's activations FIRST: the earliest matmuls
                    # need xt + the first wg columns; emitting xt after 17MB
                    # of weight DMA left the PE idle ~57us at startup.
                    tt0 = _token_tiles(C)[0][1]
                    xts0 = []
                    for d in range(DC):
                        xt_t = xpool.tile([128, tt0], bf16, tag=f"xt{d}")
                        nc.sync.dma_start(
                            xt_t[:], xt_d[d * 128:(d + 1) * 128, 0:tt0]
                        )
                        xts0.append(xt_t)
                    weights = (*emit_weights(), emit_wts())
                else:
                    xts0 = None
                wg_sb, wu_sb, wd_sb, wts_sb = weights

                for ti, (t0, TT) in enumerate(_token_tiles(CB)):
                    if ti == 0 and xts0 is not None:
                        xts = xts0
                    else:
                        xts = []
                        for d in range(DC):
                            xt_t = xpool.tile([128, TT], bf16, tag=f"xt{d}")
                            nc.sync.dma_start(
                                xt_t[:], xt_d[d * 128:(d + 1) * 128, t0:t0 + TT]
                            )
                            xts.append(xt_t)

                    # Stage 1: H^T[f, t] = silu(Wg^T x) * (Wu^T x), bf16
                    hts = []
                    for f in range(FC):
                        pg = pg_pool.tile([128, TT], f32, tag="pg")
                        pu = pu_pool.tile([128, TT], f32, tag="pu")
                        for d in range(DC):
                            off = d * F + f * 128
                            nc.tensor.matmul(
                                pg[:], wg_sb[:, off:off + 128], xts[d][:],
                                start=(d == 0), stop=(d == DC - 1),
                            )
                        for d in range(DC):
                            off = d * F + f * 128
                            nc.tensor.matmul(
                                pu[:], wu_sb[:, off:off + 128], xts[d][:],
                                start=(d == 0), stop=(d == DC - 1),
                            )
                        sg = spool.tile([128, TT], f32, tag="silu")
                        nc.scalar.activation(sg[:], pg[:], AF.Silu)
                        ht = hpool.tile([128, TT], bf16, tag=f"ht{f}")
                        nc.vector.tensor_mul(ht[:], sg[:], pu[:])
                        hts.append(ht)

                    # Stage 2: Y[t, :] = (H @ Wd) * combine_weight
                    for ts in range(TT // 128):
                        g = t0 // 128 + ts
                        for dh in range(2):
                            py = py_pool.tile([128, 512], f32, tag="py")
                            for f in range(FC):
                                nc.tensor.matmul(
                                    py[:],
                                    hts[f][:, ts * 128:(ts + 1) * 128],
                                    wd_sb[:, f * D + dh * 512:
                                          f * D + dh * 512 + 512],
                                    start=(f == 0), stop=(f == FC - 1),
                                )
                            y_sb = ypool.tile([128, 512], f32, tag="y")
                            nc.vector.tensor_scalar_mul(
                                y_sb[:], py[:], wts_sb[:, g:g + 1]
                            )
                            nc.sync.dma_start(
                                y_d[t0 + ts * 128: t0 + (ts + 1) * 128,
                                    dh * 512:(dh + 1) * 512],
                                y_sb[:],
                            )

                if not C8:
                    return
                # fp8-DoubleRow sub-tile (slots CB..C): gate/up contract
                # 256 rows per pass (d-chunk pairs 2c,2c+1) at half the PE
                # cycles; activation descales by 1/(8*256), the remaining
                # 1/2048 on u is folded into these slots' combine weights
                # (host). Stage 2 is the standard bf16 path.
                DR = mybir.MatmulPerfMode.DoubleRow
                xt8_sb = xpool.tile([128, DC * C8], fp8, tag="xt8")
                nc.sync.dma_start(xt8_sb[:], xt8_d)
                hts = []
                for f in range(FC):
                    w8g = wspool.tile([128, DC * 128], fp8, tag="w8g")
                    nc.sync.dma_start(
                        w8g[:], wg8_d[:, f * 1024:(f + 1) * 1024]
                    )
                    w8u = wspool.tile([128, DC * 128], fp8, tag="w8u")
                    nc.sync.dma_start(
                        w8u[:], wu8_d[:, f * 1024:(f + 1) * 1024]
                    )
                    pg = pg_pool.tile([128, 512], f32, tag="pg")
                    pu = pu_pool.tile([128, 512], f32, tag="pu")
                    for c in range(DC // 2):
                        rx = xt8_sb[:, 2 * c * C8:(2 * c + 2) * C8].rearrange(
                            "p (ko n) -> p ko n", ko=2
                        )
                        nc.tensor.matmul(
                            pg[:, 0:C8],
                            w8g[:, 2 * c * 128:(2 * c + 2) * 128].rearrange(
                                "p (ko m) -> p ko m", ko=2
                            ),
                            rx,
                            start=(c == 0), stop=(c == DC // 2 - 1),
                            perf_mode=DR,
                        )
                    for c in range(DC // 2):
                        rx = xt8_sb[:, 2 * c * C8:(2 * c + 2) * C8].rearrange(
                            "p (ko n) -> p ko n", ko=2
                        )
                        nc.tensor.matmul(
                            pu[:, 0:C8],
                            w8u[:, 2 * c * 128:(2 * c + 2) * 128].rearrange(
                                "p (ko m) -> p ko m", ko=2
                            ),
                            rx,
                            start=(c == 0), stop=(c == DC // 2 - 1),
                            perf_mode=DR,
                        )
                    sg = spool.tile([128, C8], f32, tag="silu")
                    nc.scalar.activation(
                        sg[:], pg[:, 0:C8], AF.Silu, scale=1.0 / 2048.0
                    )
                    ht = hpool.tile([128, C8], bf16, tag=f"ht{f}")
                    nc.vector.tensor_mul(ht[:], sg[:], pu[:, 0:C8])
                    hts.append(ht)

                for ts in range(C8 // 128):
                    g = CB // 128 + ts
                    for dh in range(2):
                        py = py_pool.tile([128, 512], f32, tag="py")
                        for f in range(FC):
                            nc.tensor.matmul(
                                py[:],
                                hts[f][:, ts * 128:(ts + 1) * 128],
                                wd_sb[:, f * D + dh * 512:
                                      f * D + dh * 512 + 512],
                                start=(f == 0), stop=(f == FC - 1),
                            )
                        y_sb = ypool.tile([128, 512], f32, tag="y")
                        nc.vector.tensor_scalar_mul(
                            y_sb[:], py[:], wts_sb[:, g:g + 1]
                        )
                        nc.sync.dma_start(
                            y_d[CB + ts * 128: CB + (ts + 1) * 128,
                                dh * 512:(dh + 1) * 512],
                            y_sb[:],
                        )

            hoist = w_outside and repeats > 1
            weights = None
            if hoist:
                weights = (*emit_weights(), emit_wts())
            if repeats > 1:
                stk.enter_context(tc.For_i(0, repeats, 1, **_LOOP_OPTS))
            for rep in range(unroll):
                emit_body(rep == 0, weights)

    nc.compile()
    return nc


_CACHE = {}


def _get_program(C):
    if C not in _CACHE:
        _CACHE[C] = _build(C)
    return _CACHE[C]


def _silu(a):
    return a / (1.0 + np.exp(-a))


def kernel(hidden_states, Wr, Wg, Wu, Wd, _timing=None):
    B, S, _ = hidden_states.shape
    T = B * S
    x = np.ascontiguousarray(
        np.asarray(hidden_states, dtype=np.float32).reshape(T, D)
    )
    Wr = np.asarray(Wr, np.float32)
    Wg = np.asarray(Wg, np.float32)
    Wu = np.asarray(Wu, np.float32)
    Wd = np.asarray(Wd, np.float32)

    idx, w = _route(x, Wr)  # [T, K]

    # Sort (token, k) pairs by expert; stable keeps deterministic layout.
    ep = idx.reshape(-1)  # expert of pair p = t*K + k
    perm = np.argsort(ep, kind="stable")
    counts = np.bincount(ep, minlength=E)
    offs = np.concatenate([[0], np.cumsum(counts)])

    # Device capacity: T*K/E is the perfectly balanced load. Tokens beyond
    # C per expert (small for near-uniform routing) run on host in fp32.
    C = T * TOPK // E
    if counts.max() > 2 * C:  # pathological skew: grow capacity instead
        C = int(-(-counts.max() // 128)) * 128 // 2 * 2

    nc = _get_program(C)

    w_flat = w.reshape(-1)
    Wg16 = Wg.astype(BF16)
    Wu16 = Wu.astype(BF16)
    Wd16 = Wd.astype(BF16)
    x16 = x.astype(BF16)

    C8 = _C8 if C == 2048 else 0
    CB = C - C8
    E4 = ml_dtypes.float8_e4m3  # TRN FP8_EXP4 (max ±240)
    if C8:
        x8 = np.clip(x * 8.0, -240, 240).astype(E4)

    in_maps = []
    placed = []  # per expert: (bf16 pairs, fp8 pairs, overflow pairs)
    for e in range(E):
        pe_all = perm[offs[e]:offs[e + 1]]
        n = len(pe_all)
        # route the n8 smallest-combine-weight pairs beyond bf16 capacity
        # to the fp8 sub-tile; anything beyond CB+C8 goes to the host path
        n8 = min(max(n - CB, 0), C8)
        if n8:
            order = np.argsort(w_flat[pe_all], kind="stable")
            m8 = np.zeros(n, dtype=bool)
            m8[order[:n8]] = True
            pe8 = pe_all[m8]
            rest = pe_all[~m8]
        else:
            pe8 = pe_all[:0]
            rest = pe_all
        peb = rest[:CB]
        pov = rest[CB:]
        placed.append((peb, pe8, pov))

        xt = np.zeros((D, C), dtype=BF16)
        xt[:, :len(peb)] = x16[peb // TOPK].T
        wts = np.zeros((C,), dtype=np.float32)
        wts[:len(peb)] = w_flat[peb]
        in_map = {
            "wg": np.ascontiguousarray(Wg16[e]),
            "wu": np.ascontiguousarray(Wu16[e]),
            "wd": np.ascontiguousarray(Wd16[e]),
            "xt": xt,
        }
        if C8:
            xt8 = np.zeros((D, C8), dtype=E4)
            xt8[:, :len(pe8)] = x8[pe8 // TOPK].T
            # u carries the 8*256 operand prescale through the (linear)
            # down-projection; fold the exact 2^-11 descale in here
            wts[CB:CB + len(pe8)] = w_flat[pe8] / 2048.0
            # partition-layout packs (see _build): xt8 [p, c*C8+n],
            # w*8 [p, f*1024 + q*128 + m]
            in_map["xt8"] = np.ascontiguousarray(
                xt8.reshape(DC, 128, C8).transpose(1, 0, 2).reshape(128, -1)
            )
            w8 = np.clip(Wg[e] * 256.0, -240, 240).astype(E4)
            in_map["wg8"] = np.ascontiguousarray(
                w8.reshape(DC, 128, FC, 128)
                .transpose(1, 2, 0, 3).reshape(128, -1)
            )
            w8 = np.clip(Wu[e] * 256.0, -240, 240).astype(E4)
            in_map["wu8"] = np.ascontiguousarray(
                w8.reshape(DC, 128, FC, 128)
                .transpose(1, 2, 0, 3).reshape(128, -1)
            )
        # pack [C] -> [128, G]: column g = weights of token group g
        in_map["wts"] = np.ascontiguousarray(wts.reshape(C // 128, 128).T)
        in_maps.append(in_map)

    try:
        res = run_bass_kernel_spmd(nc, in_maps, list(range(E)))
    except ModuleNotFoundError:
        # BASS_TRACE set but this axon client lacks the NTFF profile hook
        import os
        os.environ["BASS_NEVER_TRACE"] = "1"
        res = run_bass_kernel_spmd(nc, in_maps, list(range(E)))
    if _timing is not None:
        _timing["results"] = res

    # Host fp32 FFN for overflow pairs (beyond per-expert capacity),
    # then un-permute device outputs back to (token, k) order.
    y_pairs = np.empty((T * TOPK, D), dtype=np.float32)
    for e, (peb, pe8, pov) in enumerate(placed):
        if len(pov):
            xo = x[pov // TOPK]
            h = _silu(xo @ Wg[e]) * (xo @ Wu[e])
            y_pairs[pov] = (h @ Wd[e]) * w_flat[pov][:, None]
        y = res.results[e]["y"]
        y_pairs[peb] = y[:len(peb)]
        if len(pe8):
            y_pairs[pe8] = y[CB:CB + len(pe8)]
    out = y_pairs.reshape(T, TOPK, D).sum(axis=1)
    return out.reshape(B, S, D).astype(np.float32)



# revision 21
# speedup vs baseline: 1.0163x; 1.0163x over previous
"""MoE FFN (8 experts, top-2) on 8 Trainium2 NeuronCores.

Strategy: expert-parallel. The router (tiny: T x D @ D x E, 0.05% of the
FLOPs) runs on host, tokens are dispatched (gathered + padded) per expert
on host, and core e runs the SwiGLU FFN for expert e over its tokens:
    y = (silu(x @ Wg[e]) * (x @ Wu[e])) @ Wd[e] * combine_weight
Per-core capacity is fixed at C = T*top_k/E = 2048 token-pairs (the
perfectly balanced load); the few overflow pairs of overloaded experts
(~1.8% for near-uniform routing) are computed on host in fp32. The
per-(token,expert) outputs are un-permuted and summed over the top-2
assignments on host. All matmuls run in bf16 with fp32 PSUM accumulation
(measured 0.4% relative error end-to-end).

On-device layout per core (all stationary operands f/d-chunked to 128):
  stage 1: G^T/U^T [f,t] tiles = Wg/Wu chunk^T @ X^T, f-major so stage 2
           needs no transpose; silu+mul fused on scalar/vector engines.
  stage 2: Y [t,d] = H^T chunks^T @ Wd chunks, scaled by combine weight.

Shapes are hardcoded for B=4, S=2048, D=1024, F=2816, E=8, top_k=2.

Performance: the per-core program issues 2112 N=512 bf16 matmuls =
1.08M PE streaming cycles, the exact MAC-count floor for a 128x128
array. Cost-model sim: 466us total with PE 96.9% busy and a 99.7%-dense
PE stream (the 3% = DMA lead-in + drain tail, both amortized across
invocations). The HW/sim gap is P0 clock throttle under 8-core load
(session-dependent, ~2.0-2.2 GHz vs the 2.4 GHz peak).

Measurement notes (this shapes test.py): each For_i back-edge costs
~10-15us on HW even with staggered_reset, so benchmark programs must
unroll bodies (repeats x unroll) — the old 42x1-loop protocol
over-reported per-body time by ~2.5%. With back-edge-free bodies,
burst (42-body window) and sustained (102-body) estimates agree to
~1-2%. Hoisting the 17MB weight re-DMA out of the benchmark loop
measures identically (+-0.3%) — DMA power is not the throttle driver.

Precision: fp8e4/e5 + DoubleRow is the only sub-bf16-cycle mode on
TRN2, but both operands quantize to 3 mantissa bits: measured end-to-
end L2 error 6.5% all-fp8, 3.8% for the rank-2 half of the pairs —
over the 2e-2 gate. The shipped compromise: a 256-slot fp8 sub-tile
per core (_C8) running gate/up in DoubleRow (half the PE cycles on
12.5% of slots, down-projection bf16), fed the per-expert smallest-
combine-weight pairs so the quantization error lands where the combine
weight minimizes it. fp8 weights are streamed from HBM per f-chunk
(used once per body — no SBUF residency needed). Device-measured:
rel err 1.16e-2, ~10us/iter faster (~2%; half the ideal 22us because
DoubleRow disables FWL, leaving the 256-col stationary loads partially
LDWEIGHTS-bound at N=256 moving). Wider fp8 coverage (C8=512) would
fix the LDW ratio but pushes error to ~2e-2 = the gate. The bf16 bulk
runs at ~100% of the throttled tensor-engine roofline (a pure-matmul
probe of the same stream measures identical time).
"""

import numpy as np
import ml_dtypes

import concourse.mybir as mybir
import concourse.tile as tile
from concourse import bacc
from concourse.bass_utils import run_bass_kernel_spmd

BF16 = ml_dtypes.bfloat16

D = 1024
F = 2816
E = 8
TOPK = 2
DC = D // 128  # 8 contraction chunks for gate/up
FC = F // 128  # 22 contraction chunks for down

# Slots per core computed with fp8-e4m3 DoubleRow gate/up matmuls (the last
# C8 of the 2048): the host routes each expert's smallest-combine-weight
# pairs there, so the fp8 quantization error lands on the least-weighted
# pairs. Down-projection stays bf16. Measured end-to-end rel err ~1.3e-2
# (vs 4.1e-3 all-bf16, gate 2e-2); saves half the gate/up PE cycles on
# C8/C of the slots.
_C8 = 256

# Options for the benchmark-only repeats loop (see _build). staggered_reset
# replaces the default drain + all-engine-barrier back-edge (~2us, plus it
# serializes each iteration's weight-DMA ramp) with overlapped per-stage
# semaphore resets; measured ~10us/iter faster than the barrier back-edge.
_LOOP_OPTS: dict = {"staggered_reset": True}


def _route(x, Wr):
    """Host router matching the jax reference: softmax -> top-2 -> renorm.

    top_k on probs == top_k on logits (softmax is monotone); argsort with
    stable kind matches jax.lax.top_k's lowest-index tie-break.
    """
    logits = x @ Wr  # [T, E] fp32
    lmax = logits.max(-1, keepdims=True)
    p = np.exp(logits - lmax)
    p /= p.sum(-1, keepdims=True)
    idx = np.argsort(-p, axis=-1, kind="stable")[:, :TOPK]
    w = np.take_along_axis(p, idx, -1)
    w = w / w.sum(-1, keepdims=True)
    return idx.astype(np.int64), w.astype(np.float32)


def _token_tiles(C):
    tiles = []
    t0 = 0
    while t0 < C:
        tt = min(512, C - t0)
        tiles.append((t0, tt))
        t0 += tt
    return tiles


def _build(C, repeats=1, w_outside=False, unroll=1):
    """Build + compile the per-core expert-FFN program for capacity C.

    repeats>1 wraps the body in a hardware loop (body runs repeats*unroll
    times) — used only for wall-clock benchmarking (per-iteration time =
    delta / extra iterations). w_outside hoists the iteration-invariant
    weight + combine-weight DMAs out of the benchmark loop; unroll>1 emits
    several bodies per hardware-loop iteration (fewer back-edges).
    """
    f32 = mybir.dt.float32
    bf16 = mybir.dt.bfloat16
    AF = mybir.ActivationFunctionType

    nc = bacc.Bacc("TRN2", target_bir_lowering=False, debug=False, num_devices=E)

    wg_d = nc.dram_tensor("wg", [D, F], bf16, kind="ExternalInput").ap()
    wu_d = nc.dram_tensor("wu", [D, F], bf16, kind="ExternalInput").ap()
    wd_d = nc.dram_tensor("wd", [F, D], bf16, kind="ExternalInput").ap()
    fp8 = mybir.dt.float8e4
    G = C // 128  # token groups of 128
    C8 = _C8 if C == 2048 else 0  # fp8-DoubleRow sub-tile slots (see kernel())
    CB = C - C8
    xt_d = nc.dram_tensor("xt", [D, C], bf16, kind="ExternalInput").ap()
    # combine weights pre-packed on host as [128, G]: column g holds the
    # weights of token group g (one straight DMA instead of a 2048x4B
    # descriptor gather, which used to stall a DMA queue)
    wts_d = nc.dram_tensor("wts", [128, G], f32, kind="ExternalInput").ap()
    if C8:
        # fp8 operands for the DoubleRow sub-tile: x pre-scaled by 8,
        # gate/up weights pre-scaled by 256 (exact powers of two; the
        # 1/2048 descale folds into the activation scale and the host-
        # prepared combine weights of these slots). Host pre-packs the
        # SBUF partition layouts: xt8 [p, c*C8+n] = X8[c*128+p, n];
        # w*8 [p, f*1024 + q*128 + m] = W8[q*128+p, f*128+m].
        xt8_d = nc.dram_tensor(
            "xt8", [128, DC * C8], fp8, kind="ExternalInput"
        ).ap()
        wg8_d = nc.dram_tensor(
            "wg8", [128, FC * DC * 128], fp8, kind="ExternalInput"
        ).ap()
        wu8_d = nc.dram_tensor(
            "wu8", [128, FC * DC * 128], fp8, kind="ExternalInput"
        ).ap()
    y_d = nc.dram_tensor("y", [C, D], f32, kind="ExternalOutput").ap()

    import contextlib

    with tile.TileContext(nc) as tc:
        with contextlib.ExitStack() as stk:
            wpool = stk.enter_context(tc.tile_pool(name="weights", bufs=1))
            xpool = stk.enter_context(tc.tile_pool(name="xt", bufs=2))
            hpool = stk.enter_context(tc.tile_pool(name="ht", bufs=1))
            spool = stk.enter_context(tc.tile_pool(name="silu", bufs=3))
            ypool = stk.enter_context(tc.tile_pool(name="yout", bufs=3))
            pg_pool = stk.enter_context(tc.tile_pool(name="pg", bufs=2, space="PSUM"))
            pu_pool = stk.enter_context(tc.tile_pool(name="pu", bufs=2, space="PSUM"))
            py_pool = stk.enter_context(tc.tile_pool(name="py", bufs=4, space="PSUM"))
            if C8:
                # streamed fp8 gate/up weight columns (each [128, DC*128]
                # chunk is used once per body; no residency). bufs=3: chunks
                # f, f+1, f+2 in flight so the DMA leads the interleaved
                # consumption by two chunks.
                wspool = stk.enter_context(tc.tile_pool(name="w8", bufs=3))

            def emit_weights():
                # Resident weights: gate/up as [128, DC*F] (d-chunk major),
                # down as [128, FC*D] (f-chunk major). Column-chunked DMA so
                # the first f-chunks of stage 1 are ready before the full
                # 17MB lands.
                wg_sb = wpool.tile([128, DC * F], bf16, tag="wg")
                wu_sb = wpool.tile([128, DC * F], bf16, tag="wu")
                wd_sb = wpool.tile([128, FC * D], bf16, tag="wd")
                CCH = 4  # column chunks per [128, F] weight slice
                for c0, c1 in [
                    (F * c // CCH, F * (c + 1) // CCH) for c in range(CCH)
                ]:
                    for d in range(DC):
                        nc.sync.dma_start(
                            wg_sb[:, d * F + c0:d * F + c1],
                            wg_d[d * 128:(d + 1) * 128, c0:c1],
                        )
                    for d in range(DC):
                        nc.sync.dma_start(
                            wu_sb[:, d * F + c0:d * F + c1],
                            wu_d[d * 128:(d + 1) * 128, c0:c1],
                        )
                # Wd in two ~2.9MB DMAs (f-chunk-major per partition); the
                # first half completes early enough for tile 0's stage 2.
                wd_3d = wd_d.rearrange("(f p) d -> p f d", p=128)
                wd_sb3 = wd_sb[:].rearrange("p (f d) -> p f d", f=FC)
                for fa, fb in ((0, FC // 2), (FC // 2, FC)):
                    nc.sync.dma_start(wd_sb3[:, fa:fb], wd_3d[:, fa:fb])
                return wg_sb, wu_sb, wd_sb

            def emit_wts():
                # Combine weights, one column per 128-token group,
                # host-packed as [128, G] so this is one contiguous DMA.
                # Issued last: first use is ~90us in (stage 2, tile 0).
                wts_sb = wpool.tile([128, G], f32, tag="wts")
                nc.sync.dma_start(wts_sb[:], wts_d)
                return wts_sb

            def emit_body(first, weights):
                if weights is None:
                    # Token tile 0's activations FIRST: the earliest matmuls
                    # need xt + the first wg columns; emitting xt after 17MB
                    # of weight DMA left the PE idle ~57us at startup.
                    tt0 = _token_tiles(C)[0][1]
                    xts0 = []
                    for d in range(DC):
                        xt_t = xpool.tile([128, tt0], bf16, tag=f"xt{d}")
                        nc.sync.dma_start(
                            xt_t[:], xt_d[d * 128:(d + 1) * 128, 0:tt0]
                        )
                        xts0.append(xt_t)
                    weights = (*emit_weights(), emit_wts())
                else:
                    xts0 = None
                wg_sb, wu_sb, wd_sb, wts_sb = weights

                for ti, (t0, TT) in enumerate(_token_tiles(CB)):
                    if ti == 0 and xts0 is not None:
                        xts = xts0
                    else:
                        xts = []
                        for d in range(DC):
                            xt_t = xpool.tile([128, TT], bf16, tag=f"xt{d}")
                            nc.sync.dma_start(
                                xt_t[:], xt_d[d * 128:(d + 1) * 128, t0:t0 + TT]
                            )
                            xts.append(xt_t)

                    # Stage 1: H^T[f, t] = silu(Wg^T x) * (Wu^T x), bf16
                    hts = []
                    for f in range(FC):
                        pg = pg_pool.tile([128, TT], f32, tag="pg")
                        pu = pu_pool.tile([128, TT], f32, tag="pu")
                        for d in range(DC):
                            off = d * F + f * 128
                            nc.tensor.matmul(
                                pg[:], wg_sb[:, off:off + 128], xts[d][:],
                                start=(d == 0), stop=(d == DC - 1),
                            )
                        for d in range(DC):
                            off = d * F + f * 128
                            nc.tensor.matmul(
                                pu[:], wu_sb[:, off:off + 128], xts[d][:],
                                start=(d == 0), stop=(d == DC - 1),
                            )
                        sg = spool.tile([128, TT], f32, tag="silu")
                        nc.scalar.activation(sg[:], pg[:], AF.Silu)
                        ht = hpool.tile([128, TT], bf16, tag=f"ht{f}")
                        nc.vector.tensor_mul(ht[:], sg[:], pu[:])
                        hts.append(ht)

                    # fp8-DoubleRow stage 1 as a generator (one yield per DR
                    # matmul): interleaved 2:1 with the LAST bf16 tile's
                    # stage-2 matmuls below, so each 213ns bf16 N=512 matmul
                    # hides one 256-column DR stationary load (DoubleRow
                    # disables FWL, so back-to-back DR passes are LDWEIGHTS-
                    # bound at N=256 moving).
                    def issue_w8(f):
                        w8g = wspool.tile([128, DC * 128], fp8, tag="w8g")
                        nc.sync.dma_start(
                            w8g[:], wg8_d[:, f * 1024:(f + 1) * 1024]
                        )
                        w8u = wspool.tile([128, DC * 128], fp8, tag="w8u")
                        nc.sync.dma_start(
                            w8u[:], wu8_d[:, f * 1024:(f + 1) * 1024]
                        )
                        return w8g, w8u

                    def dr_stage1(hts8):
                        DR = mybir.MatmulPerfMode.DoubleRow
                        xt8_sb = xpool.tile([128, DC * C8], fp8, tag="xt8")
                        nc.sync.dma_start(xt8_sb[:], xt8_d)
                        w8q = [issue_w8(0), issue_w8(1)]
                        for f in range(FC):
                            if f + 2 < FC:
                                w8q.append(issue_w8(f + 2))
                            w8g, w8u = w8q.pop(0)
                            pg = pg_pool.tile([128, 512], f32, tag="pg")
                            pu = pu_pool.tile([128, 512], f32, tag="pu")
                            for w8, ps in ((w8g, pg), (w8u, pu)):
                                for c in range(DC // 2):
                                    nc.tensor.matmul(
                                        ps[:, 0:C8],
                                        w8[:, 2 * c * 128:(2 * c + 2) * 128]
                                        .rearrange("p (ko m) -> p ko m", ko=2),
                                        xt8_sb[:, 2 * c * C8:(2 * c + 2) * C8]
                                        .rearrange("p (ko n) -> p ko n", ko=2),
                                        start=(c == 0),
                                        stop=(c == DC // 2 - 1),
                                        perf_mode=DR,
                                    )
                                    yield
                            sg = spool.tile([128, C8], f32, tag="silu")
                            nc.scalar.activation(
                                sg[:], pg[:, 0:C8], AF.Silu, scale=1.0 / 2048.0
                            )
                            ht = hpool.tile([128, C8], bf16, tag=f"ht{f}")
                            nc.vector.tensor_mul(ht[:], sg[:], pu[:, 0:C8])
                            hts8.append(ht)

                    hts8 = []
                    gen = None
                    if C8 and ti == len(_token_tiles(CB)) - 1:
                        gen = dr_stage1(hts8)

                    # Stage 2: Y[t, :] = (H @ Wd) * combine_weight
                    for ts in range(TT // 128):
                        g = t0 // 128 + ts
                        for dh in range(2):
                            py = py_pool.tile([128, 512], f32, tag="py")
                            for f in range(FC):
                                nc.tensor.matmul(
                                    py[:],
                                    hts[f][:, ts * 128:(ts + 1) * 128],
                                    wd_sb[:, f * D + dh * 512:
                                          f * D + dh * 512 + 512],
                                    start=(f == 0), stop=(f == FC - 1),
                                )
                                if gen is not None:
                                    next(gen, None)
                                    next(gen, None)
                            y_sb = ypool.tile([128, 512], f32, tag="y")
                            nc.vector.tensor_scalar_mul(
                                y_sb[:], py[:], wts_sb[:, g:g + 1]
                            )
                            nc.sync.dma_start(
                                y_d[t0 + ts * 128: t0 + (ts + 1) * 128,
                                    dh * 512:(dh + 1) * 512],
                                y_sb[:],
                            )

                if not C8:
                    return
                for _ in gen:  # drain any remaining DR matmuls
                    pass
                hts = hts8

                for ts in range(C8 // 128):
                    g = CB // 128 + ts
                    for dh in range(2):
                        py = py_pool.tile([128, 512], f32, tag="py")
                        for f in range(FC):
                            nc.tensor.matmul(
                                py[:],
                                hts[f][:, ts * 128:(ts + 1) * 128],
                                wd_sb[:, f * D + dh * 512:
                                      f * D + dh * 512 + 512],
                                start=(f == 0), stop=(f == FC - 1),
                            )
                        y_sb = ypool.tile([128, 512], f32, tag="y")
                        nc.vector.tensor_scalar_mul(
                            y_sb[:], py[:], wts_sb[:, g:g + 1]
                        )
                        nc.sync.dma_start(
                            y_d[CB + ts * 128: CB + (ts + 1) * 128,
                                dh * 512:(dh + 1) * 512],
                            y_sb[:],
                        )

            hoist = w_outside and repeats > 1
            weights = None
            if hoist:
                weights = (*emit_weights(), emit_wts())
            if repeats > 1:
                stk.enter_context(tc.For_i(0, repeats, 1, **_LOOP_OPTS))
            for rep in range(unroll):
                emit_body(rep == 0, weights)

    nc.compile()
    return nc


_CACHE = {}


def _get_program(C):
    if C not in _CACHE:
        _CACHE[C] = _build(C)
    return _CACHE[C]


def _silu(a):
    return a / (1.0 + np.exp(-a))


def kernel(hidden_states, Wr, Wg, Wu, Wd, _timing=None):
    B, S, _ = hidden_states.shape
    T = B * S
    x = np.ascontiguousarray(
        np.asarray(hidden_states, dtype=np.float32).reshape(T, D)
    )
    Wr = np.asarray(Wr, np.float32)
    Wg = np.asarray(Wg, np.float32)
    Wu = np.asarray(Wu, np.float32)
    Wd = np.asarray(Wd, np.float32)

    idx, w = _route(x, Wr)  # [T, K]

    # Sort (token, k) pairs by expert; stable keeps deterministic layout.
    ep = idx.reshape(-1)  # expert of pair p = t*K + k
    perm = np.argsort(ep, kind="stable")
    counts = np.bincount(ep, minlength=E)
    offs = np.concatenate([[0], np.cumsum(counts)])

    # Device capacity: T*K/E is the perfectly balanced load. Tokens beyond
    # C per expert (small for near-uniform routing) run on host in fp32.
    C = T * TOPK // E
    if counts.max() > 2 * C:  # pathological skew: grow capacity instead
        C = int(-(-counts.max() // 128)) * 128 // 2 * 2

    nc = _get_program(C)

    w_flat = w.reshape(-1)
    Wg16 = Wg.astype(BF16)
    Wu16 = Wu.astype(BF16)
    Wd16 = Wd.astype(BF16)
    x16 = x.astype(BF16)

    C8 = _C8 if C == 2048 else 0
    CB = C - C8
    E4 = ml_dtypes.float8_e4m3  # TRN FP8_EXP4 (max ±240)
    if C8:
        x8 = np.clip(x * 8.0, -240, 240).astype(E4)

    in_maps = []
    placed = []  # per expert: (bf16 pairs, fp8 pairs, overflow pairs)
    for e in range(E):
        pe_all = perm[offs[e]:offs[e + 1]]
        n = len(pe_all)
        # route the n8 smallest-combine-weight pairs beyond bf16 capacity
        # to the fp8 sub-tile; anything beyond CB+C8 goes to the host path
        n8 = min(max(n - CB, 0), C8)
        if n8:
            order = np.argsort(w_flat[pe_all], kind="stable")
            m8 = np.zeros(n, dtype=bool)
            m8[order[:n8]] = True
            pe8 = pe_all[m8]
            rest = pe_all[~m8]
        else:
            pe8 = pe_all[:0]
            rest = pe_all
        peb = rest[:CB]
        pov = rest[CB:]
        placed.append((peb, pe8, pov))

        xt = np.zeros((D, C), dtype=BF16)
        xt[:, :len(peb)] = x16[peb // TOPK].T
        wts = np.zeros((C,), dtype=np.float32)
        wts[:len(peb)] = w_flat[peb]
        in_map = {
            "wg": np.ascontiguousarray(Wg16[e]),
            "wu": np.ascontiguousarray(Wu16[e]),
            "wd": np.ascontiguousarray(Wd16[e]),
            "xt": xt,
        }
        if C8:
            xt8 = np.zeros((D, C8), dtype=E4)
            xt8[:, :len(pe8)] = x8[pe8 // TOPK].T
            # u carries the 8*256 operand prescale through the (linear)
            # down-projection; fold the exact 2^-11 descale in here
            wts[CB:CB + len(pe8)] = w_flat[pe8] / 2048.0
            # partition-layout packs (see _build): xt8 [p, c*C8+n],
            # w*8 [p, f*1024 + q*128 + m]
            in_map["xt8"] = np.ascontiguousarray(
                xt8.reshape(DC, 128, C8).transpose(1, 0, 2).reshape(128, -1)
            )
            w8 = np.clip(Wg[e] * 256.0, -240, 240).astype(E4)
            in_map["wg8"] = np.ascontiguousarray(
                w8.reshape(DC, 128, FC, 128)
                .transpose(1, 2, 0, 3).reshape(128, -1)
            )
            w8 = np.clip(Wu[e] * 256.0, -240, 240).astype(E4)
            in_map["wu8"] = np.ascontiguousarray(
                w8.reshape(DC, 128, FC, 128)
                .transpose(1, 2, 0, 3).reshape(128, -1)
            )
        # pack [C] -> [128, G]: column g = weights of token group g
        in_map["wts"] = np.ascontiguousarray(wts.reshape(C // 128, 128).T)
        in_maps.append(in_map)

    try:
        res = run_bass_kernel_spmd(nc, in_maps, list(range(E)))
    except ModuleNotFoundError:
        # BASS_TRACE set but this axon client lacks the NTFF profile hook
        import os
        os.environ["BASS_NEVER_TRACE"] = "1"
        res = run_bass_kernel_spmd(nc, in_maps, list(range(E)))
    if _timing is not None:
        _timing["results"] = res

    # Host fp32 FFN for overflow pairs (beyond per-expert capacity),
    # then un-permute device outputs back to (token, k) order.
    y_pairs = np.empty((T * TOPK, D), dtype=np.float32)
    for e, (peb, pe8, pov) in enumerate(placed):
        if len(pov):
            xo = x[pov // TOPK]
            h = _silu(xo @ Wg[e]) * (xo @ Wu[e])
            y_pairs[pov] = (h @ Wd[e]) * w_flat[pov][:, None]
        y = res.results[e]["y"]
        y_pairs[peb] = y[:len(peb)]
        if len(pe8):
            y_pairs[pe8] = y[CB:CB + len(pe8)]
    out = y_pairs.reshape(T, TOPK, D).sum(axis=1)
    return out.reshape(B, S, D).astype(np.float32)

